# revision 10
# baseline (speedup 1.0000x reference)
"""Trainium2 Bass kernel for batched differentiable mean-variance optimization.

Problem: for each of 256 samples, solve
    min 0.5 y^T Sigma y  s.t.  mu^T y = 1, y >= 0
via 150 unrolled projected-gradient iterations (step = 1/lambda_max via power
iteration), then normalize to portfolio weights.  Pure data parallel across 8
cores (32 samples/core).

Design (v2):
- Sigma is shipped from the host as a precision pair: S1 = bf16(Sigma)
  (16 MB/core) plus S2 = fp8_e4m3((Sigma - S1) * 4096) (8 MB/core).  Both stay
  SBUF-resident for all 32 samples.
- Matvec Sigma @ y streams S1 chunks as the PE moving operand at 1 col/cycle
  with 4 samples running concurrently in distinct 32-column groups
  (tile_position), i.e. 512 cycles/sample/matvec.  Error-decay analysis shows
  bf16 matvec noise injected at PGD iter k fades by ~0.976^(150-k), so only
  the last 40 iterations add the fp8-scaled S2 correction matmuls
  (compensated product, ~fp32-quality fixed point) at 2x PE cost.
- Power iteration runs 10 unnormalized steps + Rayleigh quotient (step size
  only needs ~1e-3 accuracy).
- Projection onto {y>=0, mu@y=1}: warm-started Newton/active-set iterations
  (K=3/iter) in a dense A4 layout [64, 128]: partition = 4*sample+quarter.
  Masked sums fuse into scalar_tensor_tensor with accum_out; the 4-partition
  per-sample reduce+broadcast is one tiny PE matmul against a block G matrix.
  ScalarE (ACT) takes the PSUM drains, the relu projection, and the
  PSUM->SBUF transpose copies so DVE keeps only the tensor_tensor work.
- Two subgroups of 16 samples self-pipeline via the Tile list scheduler
  (PE matvec of one subgroup overlaps DVE projection of the other).
- Final weights w = y/sum(y) are computed from ys = -step*mu*z directly
  (per-sample positive rescale cancels in the normalization; relu/renorm
  of the reference are no-ops here since w >= 0 elementwise).
"""

import os
import numpy as np
from contextlib import ExitStack

N = 512
NCORES = 8
SPC = 32            # samples per core (all SBUF-resident)
SGN = 16            # samples per subgroup
POWER_ITERS = 10
PGD_ITERS = 150
SPLIT_FROM = 110    # first PGD iter that applies the S2 correction
NEWTON_K = 2        # warm-started Newton iters per projection
NEWTON_K0 = 8       # cold-start Newton iters for y0
S2_SCALE = 4096.0

_PROGRAM_CACHE = {}


def _build_program(pgd_iters=PGD_ITERS, split_from=SPLIT_FROM,
                   power_iters=POWER_ITERS, newton_k=NEWTON_K):
    import concourse.bacc as bacc
    import concourse.tile as tile
    from concourse import mybir

    Alu = mybir.AluOpType
    Act = mybir.ActivationFunctionType
    F32 = mybir.dt.float32
    BF16 = mybir.dt.bfloat16
    F8 = mybir.dt.float8e4

    nc = bacc.Bacc(
        "TRN2",
        target_bir_lowering=False,
        debug=False,
        enable_asserts=False,
        num_devices=NCORES,
    )

    mu_dram = nc.dram_tensor("mu_in", [SPC, N], F32, kind="ExternalInput").ap()
    s1_dram = nc.dram_tensor("s1_in", [SPC, N, N], BF16, kind="ExternalInput").ap()
    s2_dram = nc.dram_tensor("s2_in", [SPC, N, N], F8, kind="ExternalInput").ap()
    g64_dram = nc.dram_tensor("g64_in", [64, 64], F32, kind="ExternalInput").ap()
    id64_dram = nc.dram_tensor("id64_in", [64, 64], F32, kind="ExternalInput").ap()
    w_dram = nc.dram_tensor("w_out", [SPC, N], F32, kind="ExternalOutput").ap()

    stop_at = os.environ.get("KM_DBG_STOP", "full")

    with tile.TileContext(nc) as tc, ExitStack() as ctx:
        const_pool = ctx.enter_context(tc.tile_pool(name="const", bufs=1))
        sig_pool = ctx.enter_context(tc.tile_pool(name="sig", bufs=1))
        state_pool = ctx.enter_context(tc.tile_pool(name="state", bufs=1))
        stage_pool = ctx.enter_context(tc.tile_pool(name="stage", bufs=1))
        mv_pool = ctx.enter_context(tc.tile_pool(name="mv", bufs=1, space="PSUM"))
        tr_pool = ctx.enter_context(tc.tile_pool(name="tr", bufs=2, space="PSUM"))
        nw_pool = ctx.enter_context(tc.tile_pool(name="nw", bufs=1, space="PSUM"))

        g64_sb = const_pool.tile([64, 64], F32)
        nc.sync.dma_start(out=g64_sb, in_=g64_dram)
        id64_sb = const_pool.tile([64, 64], F32)
        nc.sync.dma_start(out=id64_sb, in_=id64_dram)

        # Resident Sigma: [part p, sample, chunk c, elem] = S[s][128c+p, e]
        s1_sb = sig_pool.tile([128, SPC, 4, N], BF16, tag="s1")
        s2_sb = sig_pool.tile([128, SPC, 4, N], F8, tag="s2")
        for b in range(SPC):
            nc.sync.dma_start(
                out=s1_sb[:, b], in_=s1_dram[b].rearrange("(c p) e -> p c e", p=128)
            )
        for b in range(SPC):
            nc.sync.dma_start(
                out=s2_sb[:, b], in_=s2_dram[b].rearrange("(c p) e -> p c e", p=128)
            )

        # Zero the matvec PSUM banks once so the [0:97] ACT drain copy never
        # reads uninitialized PSUM (only partitions 0,32,64,96 are written).
        mv_ps = []
        for jg in range(4):
            ps = mv_pool.tile([128, N], F32, tag=f"mv{jg}")
            nc.vector.memset(ps, 0.0)
            mv_ps.append(ps)

        class SG:
            pass

        sgs = []
        for sg in range(2):
            s = SG()
            s.idx = sg
            tg = f"g{sg}"
            s0 = sg * SGN
            P = 4 * SGN  # 64 partitions
            s.mu_rep = state_pool.tile([P, 128], F32, tag=f"{tg}_mu")
            # mu A4 layout: partition 4b+q <- mu[s0+b, 128q:128(q+1)]
            nc.sync.dma_start(out=s.mu_rep, in_=mu_dram[s0 : s0 + SGN, :])
            s.invmu = state_pool.tile([P, 128], F32, tag=f"{tg}_imu")
            nc.vector.reciprocal(s.invmu, s.mu_rep)
            s.musq = state_pool.tile([P, 128], F32, tag=f"{tg}_msq")
            nc.vector.tensor_mul(s.musq, s.mu_rep, s.mu_rep)

            s.u = state_pool.tile([P, 128], F32, tag=f"{tg}_u")
            s.r = state_pool.tile([P, 128], F32, tag=f"{tg}_r")
            s.muv = state_pool.tile([P, 128], F32, tag=f"{tg}_muv")
            s.zr = s.muv  # relu output reuses muv (dead after newton)
            s.ysf = state_pool.tile([P, 128], F32, tag=f"{tg}_ysf")
            s.pdma = state_pool.tile([P, 128], F32, tag=f"{tg}_pd")
            s.xB = state_pool.tile([128, P], BF16, tag=f"{tg}_xB")
            s.xB2 = state_pool.tile([128, P], BF16, tag=f"{tg}_xB2")
            s.ab = state_pool.tile([P, 2], F32, tag=f"{tg}_ab")
            s.nd = state_pool.tile([P, 2], F32, tag=f"{tg}_nd")
            s.neglam = state_pool.tile([P, 1], F32, tag=f"{tg}_nl")
            s.rb = state_pool.tile([P, 1], F32, tag=f"{tg}_rb")
            s.bmax = state_pool.tile([P, 1], F32, tag=f"{tg}_bm")
            s.negstep = state_pool.tile([P, 1], F32, tag=f"{tg}_ns")
            s.invnegstep = state_pool.tile([P, 1], F32, tag=f"{tg}_ins")
            sgs.append(s)

        def matvec(s, dst, late):
            """dst[A4] = matvec of current stationary x_B (+ x_B2 vs S2)."""
            for jg in range(4):
                ps = mv_ps[jg]
                # p-outer, j-inner: 4 independent col-group streams issue
                # back-to-back and run concurrently; each sample's p-chain
                # advances once per round (same-group matmuls serialize).
                for p in range(4):
                    for j in range(4):
                        b = SGN * s.idx + 4 * jg + j
                        col = 4 * (4 * jg + j)
                        nc.tensor.matmul(
                            ps[32 * j : 32 * j + 1, :],
                            s.xB[:, col + p : col + p + 1],
                            s1_sb[:, b, p, :],
                            start=(p == 0),
                            stop=(p == 3 and not late),
                            tile_position=(0, 32 * j),
                        )
                if late:
                    for p in range(4):
                        for j in range(4):
                            b = SGN * s.idx + 4 * jg + j
                            col = 4 * (4 * jg + j)
                            nc.tensor.matmul(
                                ps[32 * j : 32 * j + 1, :],
                                s.xB2[:, col + p : col + p + 1],
                                s2_sb[:, b, p, :],
                                start=False,
                                stop=(p == 3),
                                tile_position=(0, 32 * j),
                            )
                # Full-bank drain through alternating stage buffers so
                # consecutive bank drains pipeline (copy of jg+1 overlaps
                # the scatter DMA of jg).
                stage = stage_pool.tile([97, N], F32, tag=f"st{jg % 2}")
                nc.scalar.copy(stage, ps[0:97, :])
                # stage[32j, 128q+f] -> dst[16jg+4j+q, f]
                nc.sync.dma_start(
                    out=dst[16 * jg : 16 * jg + 16, :],
                    in_=stage[0:97:32, :],
                )

        def to_B(s, src_f32, make_xb2):
            trp = tr_pool.tile([128, 64], F32, tag="tr")
            nc.tensor.transpose(trp, src_f32, id64_sb)
            nc.scalar.copy(s.xB, trp)
            if make_xb2:
                nc.scalar.mul(s.xB2, s.xB, 1.0 / S2_SCALE)

        def gmm(s, rhs, out_ps, n):
            nc.tensor.matmul(
                out_ps[:, 0:n], g64_sb, rhs[:, 0:n], start=True, stop=True
            )

        def newton(s, q, wr, k_iters, scratch, guard=False):
            for _ in range(k_iters):
                nc.vector.scalar_tensor_tensor(
                    out=scratch, in0=q, scalar=s.neglam[:, 0:1], in1=wr,
                    op0=Alu.is_gt, op1=Alu.mult, accum_out=s.ab[:, 0:1],
                )
                nc.vector.scalar_tensor_tensor(
                    out=scratch, in0=q, scalar=s.neglam[:, 0:1], in1=s.musq,
                    op0=Alu.is_gt, op1=Alu.mult, accum_out=s.ab[:, 1:2],
                )
                abp = nw_pool.tile([64, 2], F32, tag=f"nw{s.idx}")
                gmm(s, s.ab, abp, 2)
                if guard:
                    nc.vector.tensor_scalar(
                        out=s.bmax, in0=abp[:, 1:2], scalar1=1e-30, scalar2=None,
                        op0=Alu.max,
                    )
                    nc.vector.reciprocal(s.rb, s.bmax)
                else:
                    nc.vector.reciprocal(s.rb, abp[:, 1:2])
                nc.vector.scalar_tensor_tensor(
                    out=s.neglam, in0=abp[:, 0:1], scalar=-1.0, in1=s.rb,
                    op0=Alu.add, op1=Alu.mult,
                )

        # ---- power iteration (unnormalized) + Rayleigh step size ----
        for s in sgs:
            nc.vector.memset(s.xB, 1.0)
        for k in range(power_iters):
            for s in sgs:
                matvec(s, s.pdma, late=False)
                if k == power_iters - 1:
                    # keep v for the Rayleigh quotient
                    nc.vector.tensor_copy(s.u, s.pdma)
                to_B(s, s.pdma, make_xb2=False)
        for s in sgs:
            matvec(s, s.muv, late=False)  # w = Sigma v
        for s in sgs:
            nc.vector.scalar_tensor_tensor(
                out=s.r, in0=s.u, scalar=0.0, in1=s.muv,
                op0=Alu.add, op1=Alu.mult, accum_out=s.nd[:, 0:1],
            )
            nc.vector.scalar_tensor_tensor(
                out=s.r, in0=s.u, scalar=0.0, in1=s.u,
                op0=Alu.add, op1=Alu.mult, accum_out=s.nd[:, 1:2],
            )
            nwp = nw_pool.tile([64, 2], F32, tag=f"nw{s.idx}")
            gmm(s, s.nd, nwp, 2)
            # negstep = -(v.v)/(v.w) = -1/lmax ; invnegstep = -(v.w)/(v.v) = -lmax
            nc.vector.reciprocal(s.rb, nwp[:, 0:1])
            nc.vector.scalar_tensor_tensor(
                out=s.negstep, in0=nwp[:, 1:2], scalar=-1.0, in1=s.rb,
                op0=Alu.mult, op1=Alu.mult,
            )
            nc.vector.reciprocal(s.bmax, nwp[:, 1:2])
            nc.vector.scalar_tensor_tensor(
                out=s.invnegstep, in0=nwp[:, 0:1], scalar=-1.0, in1=s.bmax,
                op0=Alu.mult, op1=Alu.mult,
            )

        if stop_at == "power":
            for s in sgs:
                nc.sync.dma_start(
                    out=w_dram[s.idx * SGN : (s.idx + 1) * SGN, :], in_=s.u
                )
            nc.compile()
            return nc

        # ---- y0 = project(ones): u=ones -> r=invmu, muv=mu ----
        for s in sgs:
            nc.vector.memset(s.neglam, -1e30)
            newton(s, s.invmu, s.mu_rep, NEWTON_K0, s.pdma, guard=True)
            nc.vector.tensor_scalar(
                out=s.zr, in0=s.invmu, scalar1=s.neglam[:, 0:1], scalar2=0.0,
                op0=Alu.subtract, op1=Alu.max,
            )
            nc.vector.scalar_tensor_tensor(
                out=s.ysf, in0=s.mu_rep, scalar=s.negstep[:, 0:1], in1=s.zr,
                op0=Alu.mult, op1=Alu.mult,
            )
            to_B(s, s.ysf, make_xb2=(split_from == 0))

        if stop_at == "y0":
            for s in sgs:
                nc.sync.dma_start(
                    out=w_dram[s.idx * SGN : (s.idx + 1) * SGN, :], in_=s.ysf
                )
            nc.compile()
            return nc

        # ---- PGD ----
        for k in range(pgd_iters):
            late = k >= split_from
            last = k == pgd_iters - 1
            for s in sgs:
                matvec(s, s.pdma, late=late)
            for s in sgs:
                # u = y - step*Sigma y = ysf*(-lmax) + pdma
                nc.vector.scalar_tensor_tensor(
                    out=s.u, in0=s.ysf, scalar=s.invnegstep[:, 0:1], in1=s.pdma,
                    op0=Alu.mult, op1=Alu.add,
                )
                nc.vector.tensor_mul(s.r, s.u, s.invmu)
                nc.vector.tensor_mul(s.muv, s.u, s.mu_rep)
                newton(s, s.r, s.muv, newton_k, s.pdma)
                nc.vector.tensor_scalar(
                    out=s.zr, in0=s.r, scalar1=s.neglam[:, 0:1], scalar2=0.0,
                    op0=Alu.subtract, op1=Alu.max,
                )
                nc.vector.scalar_tensor_tensor(
                    out=s.ysf, in0=s.mu_rep, scalar=s.negstep[:, 0:1], in1=s.zr,
                    op0=Alu.mult, op1=Alu.mult,
                )
                if not last:
                    to_B(s, s.ysf, make_xb2=(k + 1 >= split_from - 1))

        # ---- postprocess: w = ysf / sum(ysf)  (scale/sign cancel) ----
        for s in sgs:
            sp = s.nd
            nc.vector.tensor_scalar(
                out=s.r, in0=s.ysf, scalar1=1.0, scalar2=None,
                op0=Alu.mult, op1=Alu.add, accum_out=sp[:, 0:1],
            )
            spp = nw_pool.tile([64, 2], F32, tag=f"nw{s.idx}")
            gmm(s, sp, spp, 1)
            nc.vector.reciprocal(s.rb, spp[:, 0:1])
            wf = s.u
            nc.vector.tensor_scalar(
                out=wf, in0=s.ysf, scalar1=s.rb[:, 0:1], scalar2=None, op0=Alu.mult
            )
            nc.sync.dma_start(
                out=w_dram[s.idx * SGN : (s.idx + 1) * SGN, :], in_=wf
            )

    nc.compile()
    return nc


def _get_program():
    if "nc" not in _PROGRAM_CACHE:
        _PROGRAM_CACHE["nc"] = _build_program()
    return _PROGRAM_CACHE["nc"]


def _host_inputs(mu, sig):
    """Per-core input maps: precision-split Sigma + tiny constants."""
    import ml_dtypes

    s1 = sig.astype(ml_dtypes.bfloat16)
    s2 = ((sig - s1.astype(np.float32)) * S2_SCALE).astype(ml_dtypes.float8_e4m3fn)
    g64 = np.kron(np.eye(SGN, dtype=np.float32), np.ones((4, 4), np.float32))
    id64 = np.eye(64, dtype=np.float32)
    in_maps = []
    for c in range(NCORES):
        sl = slice(c * SPC, (c + 1) * SPC)
        in_maps.append(
            {
                "mu_in": np.ascontiguousarray(mu[sl]),
                "s1_in": np.ascontiguousarray(s1[sl]),
                "s2_in": np.ascontiguousarray(s2[sl]),
                "g64_in": g64,
                "id64_in": id64,
            }
        )
    return in_maps


def kernel(predicted_returns: np.ndarray, covariance_matrix: np.ndarray) -> np.ndarray:
    from concourse.bass_utils import run_bass_kernel_spmd

    mu = np.ascontiguousarray(predicted_returns, dtype=np.float32)
    sig = np.ascontiguousarray(covariance_matrix, dtype=np.float32)
    batch = mu.shape[0]
    assert batch == NCORES * SPC and mu.shape[1] == N

    nc = _get_program()
    in_maps = _host_inputs(mu, sig)
    res = run_bass_kernel_spmd(nc, in_maps, core_ids=list(range(NCORES)))
    out = np.concatenate([r["w_out"] for r in res.results], axis=0)
    return out.astype(np.float32)


if __name__ == "__main__":
    rng = np.random.default_rng(0)
    mu = (0.05 + 0.1 * rng.random((NCORES * SPC, N))).astype(np.float32)
    A = rng.standard_normal((4, N, N)).astype(np.float32)
    sig = np.einsum("bik,bjk->bij", A, A) / N + 0.1 * np.eye(N, dtype=np.float32)
    sig = np.tile(sig, (64, 1, 1)).astype(np.float32)
    w = kernel(mu, sig)
    print(w.shape, w.sum(axis=1)[:4])
